# revision 3
# baseline (speedup 1.0000x reference)
"""Trainium2 Bass kernel for nn_AxisAttention (sparse_attention).

Math: the reference applies softmax over a size-1 axis, so every attention
weight is exactly 1.0 and the module collapses algebraically:

    v       = g @ Wv + bv                     # [N, N, D]
    out     = g + N*(v + v^T) + ...           # ^T swaps the first two axes
            = g + N*((g + g^T) @ Wv) + 2*N*bv

h = g + g^T is SYMMETRIC in its first two axes, so u = h @ (N*Wv) only
needs to be computed for the N*(N+1)/2 upper-triangular grid tokens
(x <= y); the lower triangle is the same values mirrored.  That makes the
device work a single dense GEMM over 73,920 tokens:

    u_up = h_up @ (N*Wv)        # [73920, 512] @ [512, 512]

Sharding: the token list is split 8 ways (9,240 tokens per core, padded to
73 groups of 128).  The host does all layout work (gather upper tokens,
add the transpose, pre-transpose to feature-major k-chunks, cast bf16,
fold the N scale into Wv) so the device program is a pure weight-stationary
bf16 matmul pipeline: DMA group -> 4 accumulating matmuls -> PSUM drain
(bf16) -> DMA out.  bf16 keeps the PE at 1 cycle/row (fp32 is 4) and
halves HBM traffic; end-to-end rel err is ~4e-3 against the fp32
reference, comfortably inside the 2e-2 gate.

Host unshard mirrors u into both triangles and adds g (+2N*bv) in fp32.
"""

import os
from contextlib import ExitStack

import numpy as np
import ml_dtypes

import concourse.bass as bass
import concourse.bacc as bacc
import concourse.mybir as mybir
import concourse.tile as tile
from concourse.bass_utils import run_bass_kernel_spmd

# Problem constants (hardcoded per the harness contract).
N = 384            # grid side
D = 512            # feature dim
NCORES = 8
TP = 128           # SBUF/PSUM partitions
KC = D // TP       # 4 contraction chunks
TOK = N * (N + 1) // 2      # 73920 upper-triangular tokens
TCORE = TOK // NCORES       # 9240 tokens per core
G = -(-TCORE // TP)         # 73 groups of 128 tokens per core
TPAD = G * TP               # 9344 (104 zero-pad tokens per core)

F32 = mybir.dt.float32
BF16 = mybir.dt.bfloat16
BF16_NP = ml_dtypes.bfloat16

DEFAULT_TUNE = {
    "nb": 8,           # groups per DMA batch (amortizes ~630ns/DMA HWDGE cost)
    "ramp": (1, 1, 2, 4),  # small leading batches: the PE p-state resets to
                        # 0.65GHz on any stall, so the pipeline must fill
                        # fast and then never starve the PE
    "taper": (4, 2, 1),  # small trailing batches: the final stores are the
                         # single-shot tail; small transfers drain fast
    "bufs_h": 8,       # input staging buffers (prefetch depth, in batches)
    "bufs_u": 5,       # output staging buffers (in batches)
    "bufs_ps": 8,      # PSUM banks in rotation
    "drain_engine": "vector",   # PSUM->SBUF bf16 drain
    "store_engine": "scalar",   # out-DMA queue (Activation HWDGE), separate
                                # sequencer from the load queue (SP/sync) so
                                # stores don't head-of-line-block prefetches
}

LAST_RESULTS = None  # BassKernelResults of the most recent run (for test.py)


def _build(repeat: int = 1, tune: dict | None = None):
    """Per-core Bass/Tile program (identical on all 8 cores).

    repeat > 1 wraps the group loop in a device-side For_i redoing the
    identical (idempotent) work -- used only for timing: the slope between
    two repeat values isolates pure device time from RPC/dispatch cost.
    """
    tn = dict(DEFAULT_TUNE)
    if tune:
        tn.update(tune)
    nc = bacc.Bacc(trn_type="TRN2", target_bir_lowering=False, debug=False)

    # ht[g, k, c, t] = h[token g*128+t, feature c*128+k]  (feature-major)
    ht = nc.dram_tensor("ht", [G, TP, KC, TP], BF16, kind="ExternalInput").ap()
    # w[k, c, o] = N * Wv[c*128+k, o]
    w = nc.dram_tensor("w", [TP, KC, D], BF16, kind="ExternalInput").ap()
    # u[g, t, o] = (h @ N*Wv)[token g*128+t, o]
    u = nc.dram_tensor("u", [G, TP, D], BF16, kind="ExternalOutput").ap()

    with tile.TileContext(nc) as tc, ExitStack() as ctx:
        const = ctx.enter_context(tc.tile_pool(name="const", bufs=1))
        hp = ctx.enter_context(tc.tile_pool(name="h", bufs=tn["bufs_h"]))
        up = ctx.enter_context(tc.tile_pool(name="u", bufs=tn["bufs_u"]))
        ps = ctx.enter_context(
            tc.tile_pool(name="ps", bufs=tn["bufs_ps"], space="PSUM"))
        dr_eng = getattr(nc, tn["drain_engine"])
        st_eng = getattr(nc, tn["store_engine"])

        wsb = const.tile([TP, KC, D], BF16)
        # W rides the store queue, which is idle at startup, so the first
        # hT batch and W load in parallel; chunked so the first matmul only
        # waits on chunk 0.
        for c in range(KC):
            getattr(nc, tn["store_engine"]).dma_start(wsb[:, c, :], w[:, c, :])

        NB = tn["nb"]

        def emit_batch(g0, nb):
            hT = hp.tile([TP, NB, KC, TP], BF16, tag="hT")
            nc.sync.dma_start(hT[:, :nb], ht[g0:g0 + nb].rearrange(
                "g p c t -> p g c t"))
            usb = up.tile([TP, NB, D], BF16, tag="usb")
            for b in range(nb):
                ups = ps.tile([TP, D], F32, tag="ups")
                for c in range(KC):
                    nc.tensor.matmul(ups[:], hT[:, b, c, :], wsb[:, c, :],
                                     start=(c == 0), stop=(c == KC - 1))
                if tn["drain_engine"] == "scalar":
                    dr_eng.copy(usb[:, b, :], ups[:])
                else:
                    dr_eng.tensor_copy(usb[:, b, :], ups[:])
            st_eng.dma_start(u[g0:g0 + nb].rearrange("g t d -> t g d"),
                             usb[:, :nb])

        def batch_sizes():
            sizes, left = [], G
            for r in tn["ramp"]:
                if left <= 0:
                    break
                sizes.append(min(r, left))
                left -= sizes[-1]
            taper = [t for t in tn.get("taper", ()) if t < NB]
            while left > sum(taper):
                sizes.append(min(NB, left - sum(taper)))
                left -= sizes[-1]
            for t in taper:
                if left <= 0:
                    break
                sizes.append(min(t, left))
                left -= sizes[-1]
            return sizes

        def emit_all():
            g0 = 0
            for nb in batch_sizes():
                emit_batch(g0, nb)
                g0 += nb

        unroll = tn.get("unroll", 0)
        chunk = tn.get("chunk", 1)
        if unroll:
            for _ in range(unroll):   # sim-only: steady state w/o control flow
                emit_all()
        elif repeat > 1:
            # chunk>1 unrolls several bodies per For_i trip: the loop's
            # reset block is a cross-engine barrier (~20us on HW), so
            # amortize it over `chunk` kernel executions when timing.
            with tc.For_i(0, repeat, 1):
                for _ in range(chunk):
                    emit_all()
        else:
            emit_all()

    nc.compile()
    return nc


_BUILD_CACHE = {}


def _get_program(repeat: int = 1, tune: dict | None = None):
    key = (repeat, tuple(sorted((tune or {}).items())))
    if key not in _BUILD_CACHE:
        _BUILD_CACHE[key] = _build(repeat, tune)
    return _BUILD_CACHE[key]


_IU_CACHE = None


def _triu():
    global _IU_CACHE
    if _IU_CACHE is None:
        _IU_CACHE = np.triu_indices(N)
    return _IU_CACHE


def _shard(g, wv):
    """Full fp32 inputs -> per-core {ht, w} bf16 maps (all layout on host)."""
    iu0, iu1 = _triu()
    h_up = g[iu0, iu1, :] + g[iu1, iu0, :]          # [TOK, D] fp32
    hp = np.zeros((NCORES, TPAD, D), np.float32)
    hp[:, :TCORE, :] = h_up.reshape(NCORES, TCORE, D)
    # [core, g, t, c, k] -> [core, g, k, c, t], feature-major per group
    ht = np.ascontiguousarray(
        hp.reshape(NCORES, G, TP, KC, TP).transpose(0, 1, 4, 3, 2)
    ).astype(BF16_NP)
    wN = np.ascontiguousarray(
        (wv * np.float32(N)).reshape(KC, TP, D).transpose(1, 0, 2)
    ).astype(BF16_NP)
    return [{"ht": ht[c], "w": wN} for c in range(NCORES)], (iu0, iu1)


def _unshard(g, us, iu, bv):
    """Mirror u into both triangles, add g (+2N*bv) in fp32."""
    iu0, iu1 = iu
    u = np.stack([np.asarray(uc).reshape(TPAD, D)[:TCORE] for uc in us])
    u = u.reshape(TOK, D).astype(np.float32)
    U = np.empty((N, N, D), np.float32)
    U[iu0, iu1] = u
    U[iu1, iu0] = u      # diagonal tokens rewritten with the same value
    out = g + U
    if np.any(bv):
        out += np.float32(2 * N) * bv
    return out


def _unit_math_numpy(ht_c, wN):
    """Numpy model of one core's device program (bf16 rounding included)."""
    htok = np.ascontiguousarray(
        ht_c.transpose(0, 3, 2, 1)).reshape(TPAD, D).astype(np.float32)
    wfull = np.ascontiguousarray(
        wN.transpose(1, 0, 2)).reshape(D, D).astype(np.float32)
    return (htok @ wfull).astype(BF16_NP).reshape(G, TP, D)


def kernel(g, Wq_w, Wq_b, Wk_w, Wk_b, Wv_w, Wv_b, _backend="hw"):
    global LAST_RESULTS
    g = np.ascontiguousarray(np.asarray(g, np.float32))
    wv = np.ascontiguousarray(np.asarray(Wv_w, np.float32))
    bv = np.asarray(Wv_b, np.float32)

    in_maps, iu = _shard(g, wv)

    if _backend == "numpy":
        us = [_unit_math_numpy(m["ht"], m["w"]) for m in in_maps]
        return _unshard(g, us, iu, bv)

    nc = _get_program()
    try:
        res = run_bass_kernel_spmd(nc, in_maps, core_ids=list(range(NCORES)))
    except ModuleNotFoundError:
        # BASS_TRACE set but the axon NTFF hook module isn't present in this
        # image -- retry without tracing.
        os.environ["BASS_NEVER_TRACE"] = "1"
        res = run_bass_kernel_spmd(nc, in_maps, core_ids=list(range(NCORES)))
    LAST_RESULTS = res
    return _unshard(g, [r["u"] for r in res.results], iu, bv)
